# revision 1
# baseline (speedup 1.0000x reference)
"""Trainium2 Bass kernel for quantized dense layer with Hadamard rotations.

Math (see reference): y = (H2 @ (sq(H2@x) @ sq(w@H1)) @ H1)/(64*64) + bias,
where sq() is per-tensor symmetric int8 stochastic quantization.

Structure: Sylvester Hadamards factor as Kronecker products
(H4096 = H32 (x) H128).  The forward transform per side is a per-128-tile
fp16 PE matmul against H128 (inputs converted fp32->fp16; validated to
cause ~1.2% stochastic-rounding flips = ~0.45% operand error) plus a
cross-tile DVE butterfly in fp16 (2x DVE mode).  Quantized values (<=127)
are exact in fp16, so the core GEMM runs fp16 x fp16 -> fp32 PSUM exactly.
Stochastic rounding is rint(x*scale + (0.5 - noise)) via the fp32->int16
round-to-nearest cast, with transposed fp16 (0.5 - noise) from the host.

Sharding (8 cores): the IN axis is split 8 ways for forward transforms +
quantization.  Pre-quant PE transposes put data in the exchange layout
while the scale AllReduces are in flight.  A warmup AllReduce at t=0
absorbs the CC-entry barrier and inter-core skew.  The CC chain is
ordered AR-x, AG1(x evens), AR-w, AG2(x odds), A2A(w) via an explicit
dependency gating AG2's input DMA on the AR-w readback.  Each core
computes yr[:, Fk]; the outer H8 feature factor folds into the host-side
unshard (8x8 combine over per-core outputs).

Inverse fusion: the batch H128 factor folds into the post-GEMM PE
transpose (regular matmul against H128 — is_transpose mode silently
ignores a non-identity matrix, do not use it); the feature H4 cross-tile
factor folds into the feature H128 matmul as a 4-term PSUM accumulation
with +/-H128 stationaries.  The remaining batch H32 runs as a DVE
butterfly: bits 0-1 on rolling tiles right after the transpose (hidden
under the GEMM), bits 2-4 as a radix-8 combine at the end feeding the
output DMA (temporaries reuse dead GEMM staging buffers).

Known hardware behaviors factored in: PE runs ~1.2GHz effective under
sustained load (activity throttle); fp32 matmuls lower to 2 half-speed
passes (4 cyc/row) — avoid; fp32r fails walrus codegen; collectives
serialize on one CC stream with ~10-30us latency floors and large
run-to-run variance; scalar-engine copies offload PSUM evacuation.
"""
import sys, os
sys.path.insert(0, '/opt/trn_rl_repo')
import numpy as np

B, IN, F = 4096, 2048, 4096
NCORES = 8
CS = IN // NCORES      # 256  per-core IN slice
FS = F // NCORES       # 512  per-core feature block
BT = B // 128          # 32   batch tiles
KT = IN // 128         # 16   contraction tiles
QMAX = 127.0

_cache = {}


def _sylvester(n):
    h = np.array([[1.0]], dtype=np.float32)
    while h.shape[0] < n:
        h = np.block([[h, h], [h, -h]])
    return h


def _build():
    from concourse import bass, bacc, tile, mybir
    import concourse.bass_isa as bass_isa

    DT = mybir.dt.float32
    BF = mybir.dt.bfloat16
    FH = mybir.dt.float16
    I16 = mybir.dt.int16
    A = mybir.AluOpType
    npbf = mybir.dt.np(BF)
    nph = np.float16

    nc = bacc.Bacc("TRN2", target_bir_lowering=False, debug=False,
                   num_devices=NCORES)

    xk = nc.dram_tensor("xk", [B, CS], DT, kind="ExternalInput")
    nk = nc.dram_tensor("nk", [CS, B], FH, kind="ExternalInput")   # (0.5-noise_x)^T
    wk = nc.dram_tensor("wk", [F, CS], DT, kind="ExternalInput")   # w slice^T
    mk = nc.dram_tensor("mk", [CS, F], FH, kind="ExternalInput")   # 0.5-noise_w
    out = nc.dram_tensor("out", [FS, B], FH, kind="ExternalOutput")

    wu_i = nc.dram_tensor("wu_i", [1, 1], DT)
    wu_o = nc.dram_tensor("wu_o", [1, 1], DT, addr_space="Shared")
    sx_i = nc.dram_tensor("sx_i", [1, 1], DT)
    sx_o = nc.dram_tensor("sx_o", [1, 1], DT, addr_space="Shared")
    sw_i = nc.dram_tensor("sw_i", [1, 1], DT)
    sw_o = nc.dram_tensor("sw_o", [1, 1], DT, addr_space="Shared")
    xqc0 = nc.dram_tensor("xqc0", [128, B], FH)                    # xq^T k-half 0
    xqc1 = nc.dram_tensor("xqc1", [128, B], FH)                    # xq^T k-half 1
    xqg0 = nc.dram_tensor("xqg0", [NCORES * 128, B], FH, addr_space="Shared")
    xqg1 = nc.dram_tensor("xqg1", [NCORES * 128, B], FH, addr_space="Shared")
    wac = nc.dram_tensor("wac", [IN, FS], FH)                      # A2A contrib
    wblk = nc.dram_tensor("wblk", [IN, FS], FH)

    h128b_d = nc.inline_tensor(_sylvester(128).astype(nph), name="h128b")
    h128n_d = nc.inline_tensor((-_sylvester(128)).astype(nph), name="h128n")
    idb_d = nc.inline_tensor(np.eye(128, dtype=nph), name="idb")
    H4 = _sylvester(4)
    rg = [list(range(NCORES))]

    NB = 32 * CS  # 8192 free columns in a fwd big tile

    def butterfly(nc, bufs, T, blk0, A, lo=0, hi_s=None):
        """FWHT across the tile-index axis of big tensors [128, T*blk0]."""
        n = T.bit_length() - 1 if hi_s is None else hi_s
        for s in range(lo, n):
            cur, nxt = bufs(s)
            blk = blk0 << s
            hi = T >> (s + 1)
            for h in range(hi):
                a0 = h * 2 * blk
                a1 = a0 + blk
                nc.vector.tensor_tensor(nxt[:, a0:a0 + blk],
                                        cur[:, a0:a0 + blk],
                                        cur[:, a1:a1 + blk], op=A.add)
                nc.vector.tensor_tensor(nxt[:, a1:a1 + blk],
                                        cur[:, a0:a0 + blk],
                                        cur[:, a1:a1 + blk], op=A.subtract)

    with tile.TileContext(nc) as tc:
        with tc.tile_pool(name="consts", bufs=1) as cpool:
            h128b = cpool.tile([128, 128], FH)
            h128n = cpool.tile([128, 128], FH)
            idb = cpool.tile([128, 128], FH)
            nc.sync.dma_start(h128b[:], h128b_d[:])
            nc.sync.dma_start(h128n[:], h128n_d[:])
            nc.sync.dma_start(idb[:], idb_d[:])
            qsc = tc.alloc_tile_pool(name="qsc", bufs=1)
            wu = qsc.tile([1, 1], DT, tag="wu", name="wu")
            nc.vector.memset(wu[0:1, 0:1], 0.0)
            nc.sync.dma_start(wu_i[:], wu[0:1, 0:1])
            nc.gpsimd.collective_compute(
                "AllReduce", A.max, replica_groups=rg,
                ins=[wu_i.ap().opt()], outs=[wu_o.ap().opt()])

            # ================= forward transforms + quant =================
            with tc.tile_pool(name="fwd", bufs=2) as fp_, \
                 tc.tile_pool(name="fin", bufs=2) as fin, \
                 tc.tile_pool(name="fps", bufs=1, space="PSUM") as fps, \
                 tc.tile_pool(name="qtmp", bufs=2) as qtmp, \
                 tc.tile_pool(name="qT", bufs=3) as qTp:

                def fwd_side(src_tile_ap, ntiles, side, stages=None):
                    bigA = fp_.tile([128, NB], FH, tag="bigA",
                                    name=f"bigA{side}")
                    bigB = fp_.tile([128, NB], FH, tag="bigB",
                                    name=f"bigB{side}")
                    for o in range(ntiles):
                        t = fin.tile([128, CS], DT, tag="fin", name="fint")
                        nc.sync.dma_start(t[:], src_tile_ap(o))
                        # fp32->fp16 convert on scalar (err 2^-11, validated
                        # ~1.2% quant flips -> ~0.45% operand err); fp16 MM
                        # runs 1 cyc/row vs fp32's 2x half-speed passes
                        th = fin.tile([128, CS], FH, tag="finh", name="finth")
                        nc.vector.tensor_copy(th[:], t[:])
                        ps = fps.tile([128, CS], DT, tag="ps", name="fpst",
                                      bufs=4)
                        nc.tensor.matmul(ps[:], h128b[:], th[:], start=True,
                                         stop=True)
                        # PSUM->SBUF copies on the (otherwise idle) scalar
                        nc.scalar.copy(bigA[:, o * CS:(o + 1) * CS], ps[:])
                    bufs = (lambda s: (bigA, bigB) if s % 2 == 0
                            else (bigB, bigA))
                    if stages is None:
                        butterfly(nc, bufs, 32, CS, A)
                        return bigB  # 5 stages -> result in B
                    butterfly(nc, bufs, 32, CS, A, lo=0, hi_s=stages)
                    return (bigA, bigB)

                def finish_butterfly(wr2, done):
                    bigA, bigB = wr2
                    bufs = (lambda s: (bigA, bigB) if s % 2 == 0
                            else (bigB, bigA))
                    butterfly(nc, bufs, 32, CS, A, lo=done, hi_s=5)
                    return bigB if (5 - done) % 2 == 1 else bigA

                def scale_trigger(big, tag, cc_in, cc_out):
                    # abs-max reduce of the whole tile on GPSIMD (frees DVE
                    # for the next butterfly); scalar AllReduce follows.
                    am = qsc.tile([128, 1], DT, tag=f"am{tag}",
                                  name=f"am{tag}")
                    nc.vector.tensor_reduce(am[:], big[:],
                                            axis=mybir.AxisListType.X,
                                            op=A.max,
                                            apply_absolute_value=True)
                    red = qsc.tile([128, 1], DT, tag=f"rd{tag}",
                                   name=f"rd{tag}")
                    nc.gpsimd.partition_all_reduce(
                        red[:], am[:], channels=128,
                        reduce_op=bass_isa.ReduceOp.absmax)
                    nc.sync.dma_start(cc_in[:], red[0:1, 0:1])
                    nc.gpsimd.collective_compute(
                        "AllReduce", A.max, replica_groups=rg,
                        ins=[cc_in.ap().opt()], outs=[cc_out.ap().opt()])

                def scale_finish(tag, cc_out):
                    sg = qsc.tile([1, 1], DT, tag=f"sg{tag}",
                                  name=f"sg{tag}")
                    sgd = nc.sync.dma_start(sg[0:1, :], cc_out[:])
                    scale_finish.last_dma = sgd
                    # r = QMAX/s with one newton step
                    r0 = qsc.tile([1, 1], DT, tag=f"r0{tag}", name=f"r0{tag}")
                    nc.vector.reciprocal(r0[0:1, :], sg[0:1, :])
                    mr = qsc.tile([1, 1], DT, tag=f"mr{tag}", name=f"mr{tag}")
                    nc.vector.tensor_tensor(mr[0:1, :], sg[0:1, :],
                                            r0[0:1, :], op=A.mult)
                    tw = qsc.tile([1, 1], DT, tag=f"tw{tag}", name=f"tw{tag}")
                    nc.vector.tensor_scalar(tw[0:1, :], mr[0:1, :], -1.0, 2.0,
                                            op0=A.mult, op1=A.add)
                    r1 = qsc.tile([1, 1], DT, tag=f"r1{tag}", name=f"r1{tag}")
                    nc.vector.tensor_tensor(r1[0:1, :], r0[0:1, :],
                                            tw[0:1, :], op=A.mult)
                    r127 = qsc.tile([1, 1], DT, tag=f"rq{tag}",
                                    name=f"rq{tag}")
                    nc.vector.tensor_scalar_mul(r127[0:1, :], r1[0:1, :],
                                                QMAX)
                    rb = qsc.tile([128, 1], DT, tag=f"rb{tag}",
                                  name=f"rb{tag}")
                    nc.gpsimd.partition_broadcast(rb[:, 0:1], r127[0:1, 0:1])
                    return sg, rb

                def pre_transpose(big, ntiles, side):
                    """PE-transpose the rotated fp16 data [128, ntiles*CS]
                    into two k-half tiles [128, ntiles*128] (pre-quant; this
                    hides under the scale-AllReduce latency)."""
                    outs = [qTp.tile([128, ntiles * 128], FH, tag="qT",
                                     name=f"{side}T{h}", bufs=4)
                            for h in range(2)]
                    for o in range(ntiles):
                        for h in range(2):
                            ps = fps.tile([128, 128], FH, tag="tps",
                                          name="tpst", bufs=4)
                            nc.tensor.transpose(
                                ps[:],
                                big[:, o * CS + h * 128:o * CS +
                                    (h + 1) * 128], idb[:])
                            dst = outs[h][:, o * 128:(o + 1) * 128]
                            if (o + h) % 2:
                                nc.scalar.copy(dst, ps[:])
                            else:
                                nc.vector.tensor_copy(dst, ps[:])
                    return outs

                def quant_half(tT, rb, nz, side):
                    """stochastic-round one k-half [128, N] in final layout:
                    STT -> int16 (rint via cast), copy -> fp16."""
                    n = tT.shape[1]
                    qi = qtmp.tile([128, n], I16, tag="qi", name="qit",
                                   bufs=2)
                    nc.vector.scalar_tensor_tensor(
                        qi[:], tT[:], rb[:, 0:1], nz[:], op0=A.mult,
                        op1=A.add)
                    qh = qtmp.tile([128, n], FH, tag="qh", name="qht",
                                   bufs=2)
                    nc.vector.tensor_copy(qh[:], qi[:])
                    return qh

                # ---- forward x + quant + AG, then w + quant + A2A ----
                xrB = fwd_side(lambda o: xk[o * 128:(o + 1) * 128, :], BT,
                               "x")
                scale_trigger(xrB, "x", sx_i, sx_o)
                xrT = pre_transpose(xrB, BT, "x")
                # noise loads on the scalar DMA queue, emitted after the x
                # input DMAs so they don't compete with the forward ramp
                nzx = [qtmp.tile([128, B], FH, tag="nz", name=f"nzx{h}",
                                 bufs=4) for h in range(2)]
                for h in range(2):
                    nc.scalar.dma_start(nzx[h][:],
                                        nk[h * 128:(h + 1) * 128, :])
                nzw = [qtmp.tile([128, F], FH, tag="nz", name=f"nzw{h}",
                                 bufs=4) for h in range(2)]

                # w-side H128 matmul pipeline is emitted here (PE/scalar
                # work overlaps the x butterfly); its DVE butterfly stages
                # come after quant-x so AG1 triggers as early as possible
                wr2 = fwd_side(lambda o: wk[o * 128:(o + 1) * 128, :],
                               F // 128, "w", stages=0)
                for h in range(2):
                    nc.scalar.dma_start(nzw[h][:],
                                        mk[h * 128:(h + 1) * 128, :])

                # x quant: k-half 0 -> AG1 as soon as the scale lands
                sgx, rbx = scale_finish("x", sx_o)
                qh0 = quant_half(xrT[0], rbx, nzx[0], "x")
                nc.sync.dma_start(xqc0[:, :], qh0[:])
                nc.gpsimd.collective_compute(
                    "AllGather", A.bypass, replica_groups=rg,
                    ins=[xqc0.ap().opt()], outs=[xqg0.ap().opt()])
                qh1 = quant_half(xrT[1], rbx, nzx[1], "x")

                # finish w butterfly + its scale reduce; xqc1's DMA is
                # gated on the AR-w readback so the CC core runs the chain
                # as AG1, AR-w, AG2, A2A (no AR stuck behind 16MB gathers)
                wrB = finish_butterfly(wr2, 0)
                scale_trigger(wrB, "w", sw_i, sw_o)
                wrT = pre_transpose(wrB, F // 128, "w")
                sgw, rbw = scale_finish("w", sw_o)
                d1 = nc.sync.dma_start(xqc1[:, :], qh1[:])
                bass._bass_rust.add_dep_helper(
                    d1.ins, scale_finish.last_dma.ins, sync=True,
                    reason="AG2 trigger after AR-w completes")
                nc.gpsimd.collective_compute(
                    "AllGather", A.bypass, replica_groups=rg,
                    ins=[xqc1.ap().opt()], outs=[xqg1.ap().opt()])
                wqh = [quant_half(wrT[h], rbw, nzw[h], "w")
                       for h in range(2)]
                for a in range(NCORES):
                    for h in range(2):
                        nc.sync.dma_start(
                            wac[a * CS + h * 128:a * CS + (h + 1) * 128, :],
                            wqh[h][:, a * FS:(a + 1) * FS])
                nc.gpsimd.collective_compute(
                    "AllToAll", A.bypass, replica_groups=rg,
                    ins=[wac.ap().opt()], outs=[wblk.ap().opt()])

                # alpha = sx*sw/(QMAX^2 * 2^24)
                al = qsc.tile([1, 1], DT, tag="al", name="al")
                nc.vector.tensor_tensor(al[0:1, 0:1], sgx[0:1, 0:1],
                                        sgw[0:1, 0:1], op=A.mult)
                nc.vector.tensor_scalar_mul(
                    al[0:1, 0:1], al[0:1, 0:1],
                    float(1.0 / (QMAX * QMAX * (1 << 24))))
                alb = qsc.tile([128, 1], DT, tag="alb", name="alb")
                nc.gpsimd.partition_broadcast(alb[:, 0:1], al[0:1, 0:1])

            # ================= GEMM + fused inverse =================
            with tc.tile_pool(name="gem", bufs=1) as gem, \
                 tc.tile_pool(name="gps", bufs=1, space="PSUM") as gps, \
                 tc.tile_pool(name="inv", bufs=1) as invp:
                xs = [gem.tile([128, B], FH, tag="xs", name=f"xst{kt}",
                                bufs=KT) for kt in range(KT)]
                ws = [gem.tile([128, FS], FH, tag="ws", name=f"wst{kt}",
                               bufs=KT) for kt in range(KT)]
                for par in range(2):      # evens (AG1) first, then odds
                    for kt in range(par, KT, 2):
                        src = xqg0 if kt % 2 == 0 else xqg1
                        nc.sync.dma_start(xs[kt][:],
                                          src[(kt // 2) * 128:
                                              (kt // 2 + 1) * 128, :])
                for kt in range(KT):      # ws gated on A2A: own (scalar) queue
                    nc.scalar.dma_start(ws[kt][:],
                                        wblk[kt * 128:(kt + 1) * 128, :])

                # zz: ft'-major [128, 4*4096]; holds post-step-b data
                zz = invp.tile([128, 4 * B], FH, tag="zz", name="zz")

                # GEMM in 8 groups of 4 bo; inverse pipeline per group
                for g in range(8):
                    pss = [gps.tile([128, FS], DT, tag="gp",
                                    name=f"gpt{g}_{i}", bufs=4)
                           for i in range(4)]
                    # kt-order follows AllGather arrival: evens then odds
                    for kt in [2 * j for j in range(8)] + \
                              [2 * j + 1 for j in range(8)]:
                        for i in range(4):
                            bo = g * 4 + i
                            nc.tensor.matmul(
                                pss[i][:],
                                xs[kt][:, bo * 128:(bo + 1) * 128],
                                ws[kt][:], start=(kt == 0), stop=(kt == 15))
                    # alpha-scale PSUM->SBUF bf16 on scalar engine
                    yro = invp.tile([128, 4 * FS], FH, tag="yro", name="yro",
                                    bufs=1)
                    for i in range(4):
                        nc.scalar.activation(
                            yro[:, i * FS:(i + 1) * FS], pss[i][:],
                            mybir.ActivationFunctionType.Copy,
                            scale=alb[:, 0:1])
                    # fused transpose: [b,f] -> [f,b'] with H128 on b
                    uRA = invp.tile([128, 4 * 128 * 4], FH, tag="uRA",
                                    name="uRA", bufs=1)
                    uRB = invp.tile([128, 4 * 128 * 4], FH, tag="uRB",
                                    name="uRB", bufs=1)
                    for i in range(4):
                        psT = gps.tile([128, 512], DT, tag="tp",
                                       name=f"tpt{g}_{i}", bufs=2)
                        for ft in range(4):
                            nc.tensor.matmul(
                                psT[:, ft * 128:(ft + 1) * 128],
                                yro[:, i * FS + ft * 128:
                                    i * FS + (ft + 1) * 128],
                                h128b[:], start=True, stop=True)
                        nc.scalar.copy(uRA[:, i * 512:(i + 1) * 512],
                                       psT[:])
                    # batch-H32 bits 0,1 (distance 1,2 in bo) on the roll:
                    # layout uRA = [bo-local 4][ft 4][b' 128]
                    for h in range(2):
                        a0, a1 = h * 1024, h * 1024 + 512
                        nc.vector.tensor_tensor(uRB[:, a0:a0 + 512],
                                                uRA[:, a0:a0 + 512],
                                                uRA[:, a1:a1 + 512],
                                                op=A.add)
                        nc.vector.tensor_tensor(uRB[:, a1:a1 + 512],
                                                uRA[:, a0:a0 + 512],
                                                uRA[:, a1:a1 + 512],
                                                op=A.subtract)
                    for h in range(2):
                        a0, a1 = h * 512, h * 512 + 1024
                        nc.vector.tensor_tensor(uRA[:, a0:a0 + 512],
                                                uRB[:, a0:a0 + 512],
                                                uRB[:, a1:a1 + 512],
                                                op=A.add)
                        nc.vector.tensor_tensor(uRA[:, a1:a1 + 512],
                                                uRB[:, a0:a0 + 512],
                                                uRB[:, a1:a1 + 512],
                                                op=A.subtract)
                    # step-b: feature H128 (+H4 folded) per out-tile ft'
                    for ftp in range(4):
                        zps = gps.tile([128, 512], DT, tag="zp",
                                       name=f"zps{g}_{ftp}", bufs=2)
                        for ft in range(4):
                            st = h128b if H4[ftp, ft] > 0 else h128n
                            nc.tensor.matmul(
                                zps[:],
                                st[:],
                                uRA[:].rearrange("p (bo f b) -> p f bo b",
                                                 bo=4, f=4)[:, ft, :, :],
                                start=(ft == 0), stop=(ft == 3))
                        nc.scalar.copy(
                            zz[:, ftp * B + g * 512:
                               ftp * B + (g + 1) * 512], zps[:])

                # batch-H32 bits 2,3,4: radix-8 over the 8 chunk blocks,
                # per ft'; feeds the output DMA
                for ftp in range(4):
                    zf = zz[:, ftp * B:(ftp + 1) * B]
                    tt1 = gem.tile([128, B], FH, tag="xs", name=f"t1_{ftp}",
                                   bufs=KT)
                    tt2 = gem.tile([128, B], FH, tag="xs", name=f"t2_{ftp}",
                                   bufs=KT)
                    # bit 2: chunk pairs (2m, 2m+1)
                    for m in range(4):
                        a0, a1 = (2 * m) * 512, (2 * m + 1) * 512
                        nc.vector.tensor_tensor(tt1[:, a0:a0 + 512],
                                                zf[:, a0:a0 + 512],
                                                zf[:, a1:a1 + 512], op=A.add)
                        nc.vector.tensor_tensor(tt1[:, a1:a1 + 512],
                                                zf[:, a0:a0 + 512],
                                                zf[:, a1:a1 + 512],
                                                op=A.subtract)
                    # bit 3: distance 2 chunks
                    for m in range(2):
                        a0, a1 = (4 * m) * 512, (4 * m + 2) * 512
                        nc.vector.tensor_tensor(tt2[:, a0:a0 + 1024],
                                                tt1[:, a0:a0 + 1024],
                                                tt1[:, a1:a1 + 1024],
                                                op=A.add)
                        nc.vector.tensor_tensor(tt2[:, a1:a1 + 1024],
                                                tt1[:, a0:a0 + 1024],
                                                tt1[:, a1:a1 + 1024],
                                                op=A.subtract)
                    # bit 4: distance 4 chunks; write tt1 then DMA out
                    for h, sgn in ((0, A.add), (2048, A.subtract)):
                        nc.vector.tensor_tensor(tt1[:, h:h + 2048],
                                                tt2[:, 0:2048],
                                                tt2[:, 2048:4096], op=sgn)
                        nc.sync.dma_start(
                            out[ftp * 128:(ftp + 1) * 128, h:h + 2048],
                            tt1[:, h:h + 2048])
            qsc.release()
    nc.compile()
    return nc


def kernel(**inputs):
    from concourse.bass_utils import run_bass_kernel_spmd

    if "nc" not in _cache:
        _cache["nc"] = _build()
    nc = _cache["nc"]

    x = np.asarray(inputs["inputs"], np.float32)
    w = np.asarray(inputs["kernel"], np.float32)
    bias = np.asarray(inputs["bias"], np.float32)
    nxp = (0.5 - np.asarray(inputs["noise_x"], np.float32)).astype(np.float16)
    nwp = (0.5 - np.asarray(inputs["noise_w"], np.float32)).astype(np.float16)

    in_maps = []
    for k in range(NCORES):
        cs = slice(k * CS, (k + 1) * CS)
        in_maps.append({
            "xk": np.ascontiguousarray(x[:, cs]),
            "nk": np.ascontiguousarray(nxp[:, cs].T),
            "wk": np.ascontiguousarray(w[cs, :].T),
            "mk": np.ascontiguousarray(nwp[cs, :]),
        })

    res = run_bass_kernel_spmd(nc, in_maps, list(range(NCORES)))
    V = np.stack([np.asarray(r["out"], np.float32)
                  for r in res.results])                   # [a', g, b]
    H8 = _sylvester(8)
    yT = (H8 @ V.reshape(NCORES, -1)).reshape(F, B)        # [f, b], f=a*512+g
    y = np.ascontiguousarray(yT.T) + bias[None, :]
    return y.astype(np.float32)



# revision 5
# speedup vs baseline: 1.2704x; 1.2704x over previous
"""Trainium2 Bass kernel for quantized dense layer with Hadamard rotations.

Math (see reference): y = (H2 @ (sq(H2@x) @ sq(w@H1)) @ H1)/(64*64) + bias,
where sq() is per-tensor symmetric int8 stochastic quantization.

Structure: Sylvester Hadamards factor as Kronecker products
(H4096 = H32 (x) H128).  The forward transform per side is a per-128-tile
fp16 PE matmul against H128 (inputs converted fp32->fp16; validated to
cause ~1.2% stochastic-rounding flips = ~0.45% operand error) plus a
cross-tile DVE butterfly in fp16.  Quantized values (<=127) are exact in
fp16, so the core GEMM runs fp16 x fp16 -> fp32 PSUM exactly.  Stochastic
rounding is rint(x*scale + (0.5 - noise)) via the fp32->int16
round-to-nearest cast, narrowed to int8 for the collectives.

Sharding (8 cores): the IN axis is split 8 ways for forward transforms +
quantization.  v2 schedule: the W side runs FIRST so its scale AllReduce
and the AllToAll land early; the CC stream order is
warmup-AR, AR-w, AR-x, A2A(w int8), AG1(x evens int8), AG2(x odds int8),
so the core GEMM starts as soon as AG1 lands instead of waiting for the
whole chain.  All data collectives ship int8 (half the bytes of fp16);
int8->fp16 conversion rides the scalar/vector engines during the
PE-bound GEMM phase.

The GEMM runs in two rounds: evens k-tiles accumulate while AG2 is in
flight and are stashed to SBUF as alpha-scaled fp16; the odds round adds
the stash back via a fused scalar_tensor_tensor.  The inverse fuses the
batch H128 into the post-GEMM PE transpose and applies the batch H4
(bo bits 0-1) as a DVE roll; the feature H128 is one PE matmul per
f-tile.  The remaining inverse factors (feature H32 over core x f-tile,
batch H8 over bo-chunk) fold into the host-side unshard combine.

Known hardware behaviors factored in: PE HAM throttle (1.2 GHz cold /
2.4 GHz after ~3.4us sustained); fp32 matmuls lower to 2 half-speed
passes -- avoid; collectives serialize on one CC stream (emission order
= stream order) with ~10-30us latency floors; a warmup AllReduce at t=0
absorbs the CC-entry barrier and inter-core launch skew; scalar-engine
copies offload PSUM evacuation.
"""
import sys, os
sys.path.insert(0, '/opt/trn_rl_repo')
import numpy as np

B, IN, F = 4096, 2048, 4096
NCORES = 8
CS = IN // NCORES      # 256  per-core IN slice
FS = F // NCORES       # 512  per-core feature block
BT = B // 128          # 32   batch tiles
KT = IN // 128         # 16   contraction tiles
QMAX = 127.0

_cache = {}


def _sylvester(n):
    h = np.array([[1.0]], dtype=np.float32)
    while h.shape[0] < n:
        h = np.block([[h, h], [h, -h]])
    return h


def _build():
    from concourse import bass, bacc, tile, mybir
    import concourse.bass_isa as bass_isa

    DT = mybir.dt.float32
    FH = mybir.dt.float16
    I16 = mybir.dt.int16
    I8 = mybir.dt.int8
    A = mybir.AluOpType
    nph = np.float16

    nc = bacc.Bacc("TRN2", target_bir_lowering=False, debug=False,
                   num_devices=NCORES)

    xk = nc.dram_tensor("xk", [B, CS], DT, kind="ExternalInput")
    nk = nc.dram_tensor("nk", [CS, B], FH, kind="ExternalInput")   # (0.5-noise_x)^T
    wk = nc.dram_tensor("wk", [F, CS], DT, kind="ExternalInput")   # w slice^T
    mk = nc.dram_tensor("mk", [CS, F], FH, kind="ExternalInput")   # 0.5-noise_w
    out = nc.dram_tensor("out", [FS, B], FH, kind="ExternalOutput")

    wu_i = nc.dram_tensor("wu_i", [1, 1], DT)
    wu_o = nc.dram_tensor("wu_o", [1, 1], DT, addr_space="Shared")
    sx_i = nc.dram_tensor("sx_i", [1, 1], DT)
    sx_o = nc.dram_tensor("sx_o", [1, 1], DT, addr_space="Shared")
    sw_i = nc.dram_tensor("sw_i", [1, 1], DT)
    sw_o = nc.dram_tensor("sw_o", [1, 1], DT, addr_space="Shared")
    xqc0 = nc.dram_tensor("xqc0", [128, B], I8)                    # xq^T k-half 0
    xqc1 = nc.dram_tensor("xqc1", [128, B], I8)                    # xq^T k-half 1
    xqg0 = nc.dram_tensor("xqg0", [NCORES * 128, B], I8, addr_space="Shared")
    xqg1 = nc.dram_tensor("xqg1", [NCORES * 128, B], I8, addr_space="Shared")
    wac = nc.dram_tensor("wac", [IN, FS], I8)                      # A2A contrib
    wblk = nc.dram_tensor("wblk", [IN, FS], I8)

    h128b_d = nc.inline_tensor(_sylvester(128).astype(nph), name="h128b")
    idb_d = nc.inline_tensor(np.eye(128, dtype=nph), name="idb")
    rg = [list(range(NCORES))]

    NB = 32 * CS  # 8192 free columns in a fwd big tile

    def butterfly(nc, bufs, T, blk0, A):
        """FWHT across the tile-index axis of big tensors [128, T*blk0]."""
        n = T.bit_length() - 1
        for s in range(n):
            cur, nxt = bufs(s)
            blk = blk0 << s
            hi = T >> (s + 1)
            for h in range(hi):
                a0 = h * 2 * blk
                a1 = a0 + blk
                nc.vector.tensor_tensor(nxt[:, a0:a0 + blk],
                                        cur[:, a0:a0 + blk],
                                        cur[:, a1:a1 + blk], op=A.add)
                nc.vector.tensor_tensor(nxt[:, a1:a1 + blk],
                                        cur[:, a0:a0 + blk],
                                        cur[:, a1:a1 + blk], op=A.subtract)

    with tile.TileContext(nc) as tc:
        with tc.tile_pool(name="consts", bufs=1) as cpool:
            h128b = cpool.tile([128, 128], FH)
            idb = cpool.tile([128, 128], FH)
            nc.sync.dma_start(h128b[:], h128b_d[:])
            nc.sync.dma_start(idb[:], idb_d[:])
            qsc = tc.alloc_tile_pool(name="qsc", bufs=1)
            wu = qsc.tile([1, 1], DT, tag="wu", name="wu")
            nc.vector.memset(wu[0:1, 0:1], 0.0)
            nc.sync.dma_start(wu_i[:], wu[0:1, 0:1])
            nc.gpsimd.collective_compute(
                "AllReduce", A.max, replica_groups=rg,
                ins=[wu_i.ap().opt()], outs=[wu_o.ap().opt()])

            # ================= forward transforms + quant =================
            with tc.tile_pool(name="fwd", bufs=2) as fp_, \
                 tc.tile_pool(name="fin", bufs=4) as fin, \
                 tc.tile_pool(name="fps", bufs=1, space="PSUM") as fps, \
                 tc.tile_pool(name="qtmp", bufs=2) as qtmp, \
                 tc.tile_pool(name="qT", bufs=3) as qTp:

                def fwd_side(src_tile_ap, ntiles, side):
                    bigA = fp_.tile([128, NB], FH, tag="bigA",
                                    name=f"bigA{side}")
                    bigB = fp_.tile([128, NB], FH, tag="bigB",
                                    name=f"bigB{side}")
                    for o in range(ntiles):
                        t = fin.tile([128, CS], DT, tag="fin", name="fint")
                        nc.sync.dma_start(t[:], src_tile_ap(o))
                        # fp32->fp16 convert on scalar (err 2^-11, validated
                        # ~1.2% quant flips -> ~0.45% operand err); fp16 MM
                        # runs 1 cyc/row vs fp32's 2x half-speed passes
                        th = fin.tile([128, CS], FH, tag="finh", name="finth")
                        nc.scalar.copy(th[:], t[:])
                        ps = fps.tile([128, CS], DT, tag="ps", name="fpst",
                                      bufs=4)
                        nc.tensor.matmul(ps[:], h128b[:], th[:], start=True,
                                         stop=True)
                        # PSUM->SBUF copies on the scalar engine
                        nc.scalar.copy(bigA[:, o * CS:(o + 1) * CS], ps[:])
                    bufs = (lambda s: (bigA, bigB) if s % 2 == 0
                            else (bigB, bigA))
                    butterfly(nc, bufs, 32, CS, A)
                    return bigB  # 5 stages -> result in B

                def scale_trigger(big, tag, cc_in, cc_out):
                    # abs-max reduce of the whole tile; GPSIMD cross-partition
                    # reduce frees DVE for the next butterfly.
                    am = qsc.tile([128, 1], DT, tag=f"am{tag}",
                                  name=f"am{tag}")
                    nc.vector.tensor_reduce(am[:], big[:],
                                            axis=mybir.AxisListType.X,
                                            op=A.max,
                                            apply_absolute_value=True)
                    red = qsc.tile([128, 1], DT, tag=f"rd{tag}",
                                   name=f"rd{tag}")
                    nc.gpsimd.partition_all_reduce(
                        red[:], am[:], channels=128,
                        reduce_op=bass_isa.ReduceOp.absmax)
                    nc.sync.dma_start(cc_in[:], red[0:1, 0:1])
                    nc.gpsimd.collective_compute(
                        "AllReduce", A.max, replica_groups=rg,
                        ins=[cc_in.ap().opt()], outs=[cc_out.ap().opt()])

                def scale_finish(tag, cc_out):
                    sg = qsc.tile([1, 1], DT, tag=f"sg{tag}",
                                  name=f"sg{tag}")
                    nc.sync.dma_start(sg[0:1, :], cc_out[:])
                    # r = QMAX/s with one newton step
                    r0 = qsc.tile([1, 1], DT, tag=f"r0{tag}", name=f"r0{tag}")
                    nc.vector.reciprocal(r0[0:1, :], sg[0:1, :])
                    mr = qsc.tile([1, 1], DT, tag=f"mr{tag}", name=f"mr{tag}")
                    nc.vector.tensor_tensor(mr[0:1, :], sg[0:1, :],
                                            r0[0:1, :], op=A.mult)
                    tw = qsc.tile([1, 1], DT, tag=f"tw{tag}", name=f"tw{tag}")
                    nc.vector.tensor_scalar(tw[0:1, :], mr[0:1, :], -1.0, 2.0,
                                            op0=A.mult, op1=A.add)
                    r1 = qsc.tile([1, 1], DT, tag=f"r1{tag}", name=f"r1{tag}")
                    nc.vector.tensor_tensor(r1[0:1, :], r0[0:1, :],
                                            tw[0:1, :], op=A.mult)
                    r127 = qsc.tile([1, 1], DT, tag=f"rq{tag}",
                                    name=f"rq{tag}")
                    nc.vector.tensor_scalar_mul(r127[0:1, :], r1[0:1, :],
                                                QMAX)
                    rb = qsc.tile([128, 1], DT, tag=f"rb{tag}",
                                  name=f"rb{tag}")
                    nc.gpsimd.partition_broadcast(rb[:, 0:1], r127[0:1, 0:1])
                    return sg, rb

                def pre_transpose(big, ntiles, side):
                    """PE-transpose the rotated fp16 data [128, ntiles*CS]
                    into two k-half tiles [128, ntiles*128]; 4 blocks batch
                    into one PSUM tile so evacuation is 4x cheaper."""
                    outs = [qTp.tile([128, ntiles * 128], FH, tag="qT",
                                     name=f"{side}T{h}", bufs=4)
                            for h in range(2)]
                    for h in range(2):
                        for o4 in range(ntiles // 4):
                            ps = fps.tile([128, 512], FH, tag="tps",
                                          name="tpst", bufs=4)
                            for j in range(4):
                                o = o4 * 4 + j
                                nc.tensor.transpose(
                                    ps[:, j * 128:(j + 1) * 128],
                                    big[:, o * CS + h * 128:o * CS +
                                        (h + 1) * 128], idb[:])
                            nc.scalar.copy(
                                outs[h][:, o4 * 512:(o4 + 1) * 512], ps[:])
                    return outs

                def quant_half(tT, rb, nz, side):
                    """stochastic-round one k-half [128, N] in final layout:
                    STT -> int16 (rint via cast), narrow -> int8."""
                    n = tT.shape[1]
                    qi = qtmp.tile([128, n], I16, tag="qi", name="qit",
                                   bufs=2)
                    nc.vector.scalar_tensor_tensor(
                        qi[:], tT[:], rb[:, 0:1], nz[:], op0=A.mult,
                        op1=A.add)
                    qh = qtmp.tile([128, n], I8, tag="qh", name="qht",
                                   bufs=4)
                    nc.vector.tensor_copy(qh[:], qi[:])
                    return qh

                # ---- w side first: fwd + AR-w + quant + A2A ----
                nzw = [qtmp.tile([128, F], FH, tag="nzw", name=f"nzw{h}",
                                 bufs=2) for h in range(2)]
                for h in range(2):
                    nc.scalar.dma_start(nzw[h][:],
                                        mk[h * 128:(h + 1) * 128, :])
                wrB = fwd_side(lambda o: wk[o * 128:(o + 1) * 128, :],
                               F // 128, "w")
                scale_trigger(wrB, "w", sw_i, sw_o)

                nzx = [qtmp.tile([128, B], FH, tag="nzx", name=f"nzx{h}",
                                 bufs=2) for h in range(2)]
                for h in range(2):
                    nc.scalar.dma_start(nzx[h][:],
                                        nk[h * 128:(h + 1) * 128, :])

                # ---- x side fwd (DVE butterfly overlaps AR-w flight);
                # emitted before pre_transpose(w) so the x H128 matmuls
                # aren't stuck on the PE FIFO behind transposes that wait
                # for the w butterfly ----
                xrB = fwd_side(lambda o: xk[o * 128:(o + 1) * 128, :], BT,
                               "x")
                scale_trigger(xrB, "x", sx_i, sx_o)

                # w quant -> A2A as early as the CC stream allows
                wrT = pre_transpose(wrB, F // 128, "w")
                sgw, rbw = scale_finish("w", sw_o)
                wqh = [quant_half(wrT[h], rbw, nzw[h], "w")
                       for h in range(2)]
                for a in range(NCORES):
                    for h in range(2):
                        nc.sync.dma_start(
                            wac[a * CS + h * 128:a * CS + (h + 1) * 128, :],
                            wqh[h][:, a * FS:(a + 1) * FS])
                nc.gpsimd.collective_compute(
                    "AllToAll", A.bypass, replica_groups=rg,
                    ins=[wac.ap().opt()], outs=[wblk.ap().opt()])

                # x quant -> AG1 (evens = k-half 0), AG2 (odds)
                xrT = pre_transpose(xrB, BT, "x")
                sgx, rbx = scale_finish("x", sx_o)
                qh0 = quant_half(xrT[0], rbx, nzx[0], "x")
                nc.sync.dma_start(xqc0[:, :], qh0[:])
                nc.gpsimd.collective_compute(
                    "AllGather", A.bypass, replica_groups=rg,
                    ins=[xqc0.ap().opt()], outs=[xqg0.ap().opt()])
                qh1 = quant_half(xrT[1], rbx, nzx[1], "x")
                nc.sync.dma_start(xqc1[:, :], qh1[:])
                nc.gpsimd.collective_compute(
                    "AllGather", A.bypass, replica_groups=rg,
                    ins=[xqc1.ap().opt()], outs=[xqg1.ap().opt()])

                # alpha = sx*sw/(QMAX^2 * 2^24)
                al = qsc.tile([1, 1], DT, tag="al", name="al")
                nc.vector.tensor_tensor(al[0:1, 0:1], sgx[0:1, 0:1],
                                        sgw[0:1, 0:1], op=A.mult)
                nc.vector.tensor_scalar_mul(
                    al[0:1, 0:1], al[0:1, 0:1],
                    float(1.0 / (QMAX * QMAX * (1 << 24))))
                alb = qsc.tile([128, 1], DT, tag="alb", name="alb")
                nc.gpsimd.partition_broadcast(alb[:, 0:1], al[0:1, 0:1])

            # ================= GEMM + fused inverse =================
            with tc.tile_pool(name="gem", bufs=1) as gem, \
                 tc.tile_pool(name="g8", bufs=2) as g8, \
                 tc.tile_pool(name="gps", bufs=1, space="PSUM") as gps, \
                 tc.tile_pool(name="inv", bufs=1) as invp:
                # int8 staging rotates; fp16 tiles persist through the GEMM
                xs = [gem.tile([128, B], FH, tag="xs", name=f"xst{kt}",
                               bufs=KT) for kt in range(KT)]
                ws = [gem.tile([128, FS], FH, tag="ws", name=f"wst{kt}",
                               bufs=KT) for kt in range(KT)]
                for kt in range(KT):      # ws gated on A2A: own (scalar) queue
                    w8 = g8.tile([128, FS], I8, tag="w8", name=f"w8_{kt}",
                                 bufs=2)
                    nc.scalar.dma_start(w8[:],
                                        wblk[kt * 128:(kt + 1) * 128, :])
                    nc.vector.tensor_copy(ws[kt][:], w8[:])
                for par in range(2):      # evens (AG1) first, then odds
                    src = xqg0 if par == 0 else xqg1
                    for j in range(NCORES):
                        kt = 2 * j + par
                        x8 = g8.tile([128, B], I8, tag="x8", name=f"x8_{kt}",
                                     bufs=2)
                        nc.sync.dma_start(x8[:],
                                          src[j * 128:(j + 1) * 128, :])
                        # int8->fp16 split across scalar and vector engines
                        if j % 2 == 0:
                            nc.scalar.copy(xs[kt][:], x8[:])
                        else:
                            nc.vector.tensor_copy(xs[kt][:], x8[:])

                # evens-round stash: alpha-scaled fp16 partials [128,32*512]
                stash = invp.tile([128, 32 * FS], FH, tag="stash",
                                  name="stash")
                for g in range(8):
                    pss = [gps.tile([128, FS], DT, tag="gp",
                                    name=f"gpe{g}_{i}", bufs=4)
                           for i in range(4)]
                    for kt in range(0, KT, 2):
                        for i in range(4):
                            bo = g * 4 + i
                            nc.tensor.matmul(
                                pss[i][:],
                                xs[kt][:, bo * 128:(bo + 1) * 128],
                                ws[kt][:], start=(kt == 0), stop=(kt == 14))
                    for i in range(4):
                        bo = g * 4 + i
                        nc.scalar.activation(
                            stash[:, bo * FS:(bo + 1) * FS], pss[i][:],
                            mybir.ActivationFunctionType.Copy,
                            scale=alb[:, 0:1])

                # odds round + inverse pipeline per group
                for g in range(8):
                    pss = [gps.tile([128, FS], DT, tag="gp",
                                    name=f"gpo{g}_{i}", bufs=4)
                           for i in range(4)]
                    for kt in range(1, KT, 2):
                        for i in range(4):
                            bo = g * 4 + i
                            nc.tensor.matmul(
                                pss[i][:],
                                xs[kt][:, bo * 128:(bo + 1) * 128],
                                ws[kt][:], start=(kt == 1), stop=(kt == 15))
                    # yr = alpha*psum + stash  (fused on DVE, fp16 out)
                    yro = invp.tile([128, 4 * FS], FH, tag="yro", name="yro",
                                    bufs=2)
                    for i in range(4):
                        bo = g * 4 + i
                        nc.vector.scalar_tensor_tensor(
                            yro[:, i * FS:(i + 1) * FS], pss[i][:],
                            alb[:, 0:1], stash[:, bo * FS:(bo + 1) * FS],
                            op0=A.mult, op1=A.add)
                    # fused transpose: [b,f] -> [f,b'] with H128 on b
                    uRA = invp.tile([128, 4 * 128 * 4], FH, tag="uRA",
                                    name="uRA", bufs=1)
                    uRB = invp.tile([128, 4 * 128 * 4], FH, tag="uRB",
                                    name="uRB", bufs=1)
                    for i in range(4):
                        psT = gps.tile([128, 512], DT, tag="tp",
                                       name=f"tpt{g}_{i}", bufs=2)
                        for ft in range(4):
                            nc.tensor.matmul(
                                psT[:, ft * 128:(ft + 1) * 128],
                                yro[:, i * FS + ft * 128:
                                    i * FS + (ft + 1) * 128],
                                h128b[:], start=True, stop=True)
                        nc.scalar.copy(uRA[:, i * 512:(i + 1) * 512],
                                       psT[:])
                    # batch-H4 (bo bits 0,1; distance 1,2 in i) on the roll:
                    # layout uRA = [bo-local 4][ft 4][b' 128]
                    for h in range(2):
                        a0, a1 = h * 1024, h * 1024 + 512
                        nc.vector.tensor_tensor(uRB[:, a0:a0 + 512],
                                                uRA[:, a0:a0 + 512],
                                                uRA[:, a1:a1 + 512],
                                                op=A.add)
                        nc.vector.tensor_tensor(uRB[:, a1:a1 + 512],
                                                uRA[:, a0:a0 + 512],
                                                uRA[:, a1:a1 + 512],
                                                op=A.subtract)
                    for h in range(2):
                        a0, a1 = h * 512, h * 512 + 1024
                        nc.vector.tensor_tensor(uRA[:, a0:a0 + 512],
                                                uRB[:, a0:a0 + 512],
                                                uRB[:, a1:a1 + 512],
                                                op=A.add)
                        nc.vector.tensor_tensor(uRA[:, a1:a1 + 512],
                                                uRB[:, a0:a0 + 512],
                                                uRB[:, a1:a1 + 512],
                                                op=A.subtract)
                    # feature H128 per out f-tile; H32/H8 fold into host
                    for ft in range(4):
                        zps = gps.tile([128, 512], DT, tag="zp",
                                       name=f"zps{g}_{ft}", bufs=2)
                        nc.tensor.matmul(
                            zps[:],
                            h128b[:],
                            uRA[:].rearrange("p (bo f b) -> p f bo b",
                                             bo=4, f=4)[:, ft, :, :],
                            start=True, stop=True)
                        ostg = invp.tile([128, 512], FH, tag="ostg",
                                         name=f"ostg{g}_{ft}", bufs=2)
                        nc.scalar.copy(ostg[:], zps[:])
                        nc.sync.dma_start(
                            out[ft * 128:(ft + 1) * 128,
                                g * 512:(g + 1) * 512], ostg[:])
            qsc.release()
    nc.compile()
    return nc


def kernel(**inputs):
    from concourse.bass_utils import run_bass_kernel_spmd

    if "nc" not in _cache:
        _cache["nc"] = _build()
    nc = _cache["nc"]

    x = np.asarray(inputs["inputs"], np.float32)
    w = np.asarray(inputs["kernel"], np.float32)
    bias = np.asarray(inputs["bias"], np.float32)
    nxp = (0.5 - np.asarray(inputs["noise_x"], np.float32)).astype(np.float16)
    nwp = (0.5 - np.asarray(inputs["noise_w"], np.float32)).astype(np.float16)

    in_maps = []
    for k in range(NCORES):
        cs = slice(k * CS, (k + 1) * CS)
        in_maps.append({
            "xk": np.ascontiguousarray(x[:, cs]),
            "nk": np.ascontiguousarray(nxp[:, cs].T),
            "wk": np.ascontiguousarray(w[cs, :].T),
            "mk": np.ascontiguousarray(nwp[cs, :]),
        })

    res = run_bass_kernel_spmd(nc, in_maps, list(range(NCORES)))
    V = np.stack([np.asarray(r["out"], np.float32)
                  for r in res.results])                   # [a, 4ft*128, B]
    H32 = _sylvester(32)
    H8 = _sylvester(8)
    yT = (H32 @ V.reshape(NCORES * 4, -1)).reshape(F, 8, 512)  # feature H32
    yT = np.einsum('gc,fcb->fgb', H8, yT).reshape(F, B)        # batch H8
    y = np.ascontiguousarray(yT.T) + bias[None, :]
    return y.astype(np.float32)


# revision 8
# speedup vs baseline: 1.3400x; 1.0548x over previous
"""Trainium2 Bass kernel for quantized dense layer with Hadamard rotations.

Math (see reference): y = (H2 @ (sq(H2@x) @ sq(w@H1)) @ H1)/(64*64) + bias,
where sq() is per-tensor symmetric int8 stochastic quantization.

Structure: Sylvester Hadamards factor as Kronecker products
(H4096 = H32 (x) H128).  The forward transform per side is a per-128-tile
fp16 PE matmul against H128 (inputs converted fp32->fp16; validated to
cause ~1.2% stochastic-rounding flips = ~0.45% operand error) plus a
cross-tile DVE butterfly in fp16.  Quantized values (<=127) are exact in
fp16, so the core GEMM runs fp16 x fp16 -> fp32 PSUM exactly.  Stochastic
rounding is rint(x*scale + (0.5 - noise)) via the fp32->int16
round-to-nearest cast, narrowed to int8 for the collectives.

Sharding (8 cores): the IN axis is split 8 ways for forward transforms +
quantization.  v2 schedule: the W side runs FIRST so its scale AllReduce
and the AllToAll land early; the CC stream order is
warmup-AR, AR-w, AR-x, A2A(w int8), AG1(x evens int8), AG2(x odds int8),
so the core GEMM starts as soon as AG1 lands instead of waiting for the
whole chain.  All data collectives ship int8 (half the bytes of fp16);
int8->fp16 conversion rides the scalar/vector engines during the
PE-bound GEMM phase.

The GEMM runs in two rounds: evens k-tiles accumulate while AG2 is in
flight and are stashed to SBUF as alpha-scaled fp16; the odds round adds
the stash back via a fused scalar_tensor_tensor.  The inverse fuses the
batch H128 into the post-GEMM PE transpose and applies the batch H4
(bo bits 0-1) as a DVE roll; the feature H128 is one PE matmul per
f-tile.  The remaining inverse factors (feature H32 over core x f-tile,
batch H8 over bo-chunk) fold into the host-side unshard combine.

Known hardware behaviors factored in: PE HAM throttle (1.2 GHz cold /
2.4 GHz after ~3.4us sustained); fp32 matmuls lower to 2 half-speed
passes -- avoid; collectives serialize on one CC stream (emission order
= stream order) with ~10-30us latency floors; a warmup AllReduce at t=0
absorbs the CC-entry barrier and inter-core launch skew; scalar-engine
copies offload PSUM evacuation.
"""
import sys, os
sys.path.insert(0, '/opt/trn_rl_repo')
import numpy as np

B, IN, F = 4096, 2048, 4096
NCORES = 8
CS = IN // NCORES      # 256  per-core IN slice
FS = F // NCORES       # 512  per-core feature block
BT = B // 128          # 32   batch tiles
KT = IN // 128         # 16   contraction tiles
QMAX = 127.0

_cache = {}


def _sylvester(n):
    h = np.array([[1.0]], dtype=np.float32)
    while h.shape[0] < n:
        h = np.block([[h, h], [h, -h]])
    return h


def _build():
    from concourse import bass, bacc, tile, mybir
    import concourse.bass_isa as bass_isa

    DT = mybir.dt.float32
    FH = mybir.dt.float16
    I16 = mybir.dt.int16
    I8 = mybir.dt.int8
    A = mybir.AluOpType
    nph = np.float16

    nc = bacc.Bacc("TRN2", target_bir_lowering=False, debug=False,
                   num_devices=NCORES)

    xk = nc.dram_tensor("xk", [B, CS], FH, kind="ExternalInput")
    nk = nc.dram_tensor("nk", [CS, B], FH, kind="ExternalInput")   # (0.5-noise_x)^T
    wk = nc.dram_tensor("wk", [F, CS], FH, kind="ExternalInput")   # w slice^T
    mk = nc.dram_tensor("mk", [CS, F], FH, kind="ExternalInput")   # 0.5-noise_w
    out = nc.dram_tensor("out", [FS, B], FH, kind="ExternalOutput")

    wu_i = nc.dram_tensor("wu_i", [1, 1], DT)
    wu_o = nc.dram_tensor("wu_o", [1, 1], DT, addr_space="Shared")
    s2_i = nc.dram_tensor("s2_i", [1, 2], DT)
    s2_o = nc.dram_tensor("s2_o", [1, 2], DT, addr_space="Shared")
    xqc0 = nc.dram_tensor("xqc0", [128, B], I8)                    # xq^T k-half 0
    xqc1 = nc.dram_tensor("xqc1", [128, B], I8)                    # xq^T k-half 1
    xqg0 = nc.dram_tensor("xqg0", [NCORES * 128, B], I8, addr_space="Shared")
    xqg1 = nc.dram_tensor("xqg1", [NCORES * 128, B], I8, addr_space="Shared")
    wac = nc.dram_tensor("wac", [IN, FS], I8)                      # A2A contrib
    wblk = nc.dram_tensor("wblk", [IN, FS], I8)

    h128b_d = nc.inline_tensor(_sylvester(128).astype(nph), name="h128b")
    idb_d = nc.inline_tensor(np.eye(128, dtype=nph), name="idb")
    rg = [list(range(NCORES))]

    NB = 32 * CS  # 8192 free columns in a fwd big tile

    def butterfly(nc, bufs, T, blk0, A):
        """FWHT across the tile-index axis of big tensors [128, T*blk0]."""
        n = T.bit_length() - 1
        for s in range(n):
            cur, nxt = bufs(s)
            blk = blk0 << s
            hi = T >> (s + 1)
            for h in range(hi):
                a0 = h * 2 * blk
                a1 = a0 + blk
                nc.vector.tensor_tensor(nxt[:, a0:a0 + blk],
                                        cur[:, a0:a0 + blk],
                                        cur[:, a1:a1 + blk], op=A.add)
                nc.vector.tensor_tensor(nxt[:, a1:a1 + blk],
                                        cur[:, a0:a0 + blk],
                                        cur[:, a1:a1 + blk], op=A.subtract)

    with tile.TileContext(nc) as tc:
        with tc.tile_pool(name="consts", bufs=1) as cpool:
            h128b = cpool.tile([128, 128], FH)
            idb = cpool.tile([128, 128], FH)
            nc.sync.dma_start(h128b[:], h128b_d[:])
            nc.sync.dma_start(idb[:], idb_d[:])
            qsc = tc.alloc_tile_pool(name="qsc", bufs=1)
            wu = qsc.tile([1, 1], DT, tag="wu", name="wu")
            nc.vector.memset(wu[0:1, 0:1], 0.0)
            nc.sync.dma_start(wu_i[:], wu[0:1, 0:1])
            nc.gpsimd.collective_compute(
                "AllReduce", A.max, replica_groups=rg,
                ins=[wu_i.ap().opt()], outs=[wu_o.ap().opt()])

            # ================= forward transforms + quant =================
            with tc.tile_pool(name="fwd", bufs=2) as fp_, \
                 tc.tile_pool(name="fin", bufs=4) as fin, \
                 tc.tile_pool(name="fps", bufs=1, space="PSUM") as fps, \
                 tc.tile_pool(name="qtmp", bufs=2) as qtmp, \
                 tc.tile_pool(name="qT", bufs=3) as qTp:

                def fwd_side(src_tile_ap, ntiles, side):
                    bigA = fp_.tile([128, NB], FH, tag="bigA",
                                    name=f"bigA{side}")
                    bigB = fp_.tile([128, NB], FH, tag="bigB",
                                    name=f"bigB{side}")
                    for o in range(ntiles):
                        # inputs arrive fp16 from the host (err 2^-11,
                        # validated ~1.2% quant flips -> ~0.45% operand err);
                        # fp16 MM runs 1 cyc/row vs fp32's 2x half-speed
                        th = fin.tile([128, CS], FH, tag="finh", name="finth")
                        nc.sync.dma_start(th[:], src_tile_ap(o))
                        ps = fps.tile([128, CS], DT, tag="ps", name="fpst",
                                      bufs=4)
                        nc.tensor.matmul(ps[:], h128b[:], th[:], start=True,
                                         stop=True)
                        # PSUM->SBUF copies on the scalar engine
                        nc.scalar.copy(bigA[:, o * CS:(o + 1) * CS], ps[:])
                    bufs = (lambda s: (bigA, bigB) if s % 2 == 0
                            else (bigB, bigA))
                    butterfly(nc, bufs, 32, CS, A)
                    return bigB  # 5 stages -> result in B

                def scale_trigger(big, tag, col):
                    # abs-max reduce of the whole tile; GPSIMD cross-partition
                    # reduce frees DVE for the next butterfly.
                    am = qsc.tile([128, 1], DT, tag=f"am{tag}",
                                  name=f"am{tag}")
                    nc.vector.tensor_reduce(am[:], big[:],
                                            axis=mybir.AxisListType.X,
                                            op=A.max,
                                            apply_absolute_value=True)
                    red = qsc.tile([128, 1], DT, tag=f"rd{tag}",
                                   name=f"rd{tag}")
                    nc.gpsimd.partition_all_reduce(
                        red[:], am[:], channels=128,
                        reduce_op=bass_isa.ReduceOp.absmax)
                    nc.sync.dma_start(s2_i[0:1, col:col + 1], red[0:1, 0:1])

                def scale_finish(tag, col):
                    sg = qsc.tile([1, 1], DT, tag=f"sg{tag}",
                                  name=f"sg{tag}")
                    nc.sync.dma_start(sg[0:1, :], s2_o[0:1, col:col + 1])
                    # r = QMAX/s (hardware iterative divide is accurate; a
                    # scale off by 2^-23 shifts ~no stochastic decisions)
                    r0 = qsc.tile([1, 1], DT, tag=f"r0{tag}", name=f"r0{tag}")
                    nc.vector.reciprocal(r0[0:1, :], sg[0:1, :])
                    r127 = qsc.tile([1, 1], DT, tag=f"rq{tag}",
                                    name=f"rq{tag}")
                    nc.vector.tensor_scalar_mul(r127[0:1, :], r0[0:1, :],
                                                QMAX)
                    rb = qsc.tile([128, 1], DT, tag=f"rb{tag}",
                                  name=f"rb{tag}")
                    nc.gpsimd.partition_broadcast(rb[:, 0:1], r127[0:1, 0:1])
                    return sg, rb

                def pre_transpose(big, ntiles, side):
                    """PE-transpose the rotated fp16 data [128, ntiles*CS]
                    into two k-half tiles [128, ntiles*128]; 4 blocks batch
                    into one PSUM tile so evacuation is 4x cheaper."""
                    outs = [qTp.tile([128, ntiles * 128], FH, tag="qT",
                                     name=f"{side}T{h}", bufs=4)
                            for h in range(2)]
                    for h in range(2):
                        for o4 in range(ntiles // 4):
                            ps = fps.tile([128, 512], FH, tag="tps",
                                          name="tpst", bufs=4)
                            for j in range(4):
                                o = o4 * 4 + j
                                nc.tensor.transpose(
                                    ps[:, j * 128:(j + 1) * 128],
                                    big[:, o * CS + h * 128:o * CS +
                                        (h + 1) * 128], idb[:])
                            nc.scalar.copy(
                                outs[h][:, o4 * 512:(o4 + 1) * 512], ps[:])
                    return outs

                def quant_half(tT, rb, nz, side):
                    """stochastic-round one k-half [128, N] in final layout:
                    STT -> int16 (rint via cast), narrow -> int8."""
                    n = tT.shape[1]
                    qi = qtmp.tile([128, n], I16, tag="qi", name="qit",
                                   bufs=2)
                    nc.vector.scalar_tensor_tensor(
                        qi[:], tT[:], rb[:, 0:1], nz[:], op0=A.mult,
                        op1=A.add)
                    qh = qtmp.tile([128, n], I8, tag="qh", name="qht",
                                   bufs=4)
                    nc.vector.tensor_copy(qh[:], qi[:])
                    return qh

                # ---- w side first: fwd + AR-w + quant + A2A ----
                nzw = [qtmp.tile([128, F], FH, tag="nzw", name=f"nzw{h}",
                                 bufs=2) for h in range(2)]
                for h in range(2):
                    nc.scalar.dma_start(nzw[h][:],
                                        mk[h * 128:(h + 1) * 128, :])
                wrB = fwd_side(lambda o: wk[o * 128:(o + 1) * 128, :],
                               F // 128, "w")
                scale_trigger(wrB, "w", 1)

                nzx = [qtmp.tile([128, B], FH, tag="nzx", name=f"nzx{h}",
                                 bufs=2) for h in range(2)]
                for h in range(2):
                    nc.scalar.dma_start(nzx[h][:],
                                        nk[h * 128:(h + 1) * 128, :])

                # ---- x side fwd (DVE butterfly overlaps AR-w flight);
                # emitted before pre_transpose(w) so the x H128 matmuls
                # aren't stuck on the PE FIFO behind transposes that wait
                # for the w butterfly ----
                xrB = fwd_side(lambda o: xk[o * 128:(o + 1) * 128, :], BT,
                               "x")
                scale_trigger(xrB, "x", 0)
                # ONE AllReduce for both scales (saves a ~20us stream slot)
                nc.gpsimd.collective_compute(
                    "AllReduce", A.max, replica_groups=rg,
                    ins=[s2_i.ap().opt()], outs=[s2_o.ap().opt()])

                wrT = pre_transpose(wrB, F // 128, "w")
                xrT = pre_transpose(xrB, BT, "x")

                # x quant -> AG1 (evens = k-half 0) first on the stream
                sgx, rbx = scale_finish("x", 0)
                sgw, rbw = scale_finish("w", 1)
                qh0 = quant_half(xrT[0], rbx, nzx[0], "x")
                nc.sync.dma_start(xqc0[:, :], qh0[:])
                nc.gpsimd.collective_compute(
                    "AllGather", A.bypass, replica_groups=rg,
                    ins=[xqc0.ap().opt()], outs=[xqg0.ap().opt()])

                # w quant -> A2A
                wqh = [quant_half(wrT[h], rbw, nzw[h], "w")
                       for h in range(2)]
                for a in range(NCORES):
                    for h in range(2):
                        nc.sync.dma_start(
                            wac[a * CS + h * 128:a * CS + (h + 1) * 128, :],
                            wqh[h][:, a * FS:(a + 1) * FS])
                nc.gpsimd.collective_compute(
                    "AllToAll", A.bypass, replica_groups=rg,
                    ins=[wac.ap().opt()], outs=[wblk.ap().opt()])

                # x odds -> AG2
                qh1 = quant_half(xrT[1], rbx, nzx[1], "x")
                nc.sync.dma_start(xqc1[:, :], qh1[:])
                nc.gpsimd.collective_compute(
                    "AllGather", A.bypass, replica_groups=rg,
                    ins=[xqc1.ap().opt()], outs=[xqg1.ap().opt()])

                # alpha = sx*sw/(QMAX^2 * 2^24)
                al = qsc.tile([1, 1], DT, tag="al", name="al")
                nc.vector.tensor_tensor(al[0:1, 0:1], sgx[0:1, 0:1],
                                        sgw[0:1, 0:1], op=A.mult)
                nc.vector.tensor_scalar_mul(
                    al[0:1, 0:1], al[0:1, 0:1],
                    float(1.0 / (QMAX * QMAX * (1 << 24))))
                alb = qsc.tile([128, 1], DT, tag="alb", name="alb")
                nc.gpsimd.partition_broadcast(alb[:, 0:1], al[0:1, 0:1])

            # ================= GEMM + fused inverse =================
            with tc.tile_pool(name="gem", bufs=1) as gem, \
                 tc.tile_pool(name="g8", bufs=2) as g8, \
                 tc.tile_pool(name="gps", bufs=1, space="PSUM") as gps, \
                 tc.tile_pool(name="inv", bufs=1) as invp:
                # int8 staging rotates; fp16 tiles persist through the GEMM
                xs = [gem.tile([128, B], FH, tag="xs", name=f"xst{kt}",
                               bufs=KT) for kt in range(KT)]
                ws = [gem.tile([128, FS], FH, tag="ws", name=f"wst{kt}",
                               bufs=KT) for kt in range(KT)]
                for kt in range(KT):      # ws gated on A2A: own (scalar) queue
                    w8 = g8.tile([128, FS], I8, tag="w8", name=f"w8_{kt}",
                                 bufs=2)
                    nc.scalar.dma_start(w8[:],
                                        wblk[kt * 128:(kt + 1) * 128, :])
                    nc.vector.tensor_copy(ws[kt][:], w8[:])
                for par in range(2):      # evens (AG1) first, then odds
                    src = xqg0 if par == 0 else xqg1
                    for j in range(NCORES):
                        kt = 2 * j + par
                        x8 = g8.tile([128, B], I8, tag="x8", name=f"x8_{kt}",
                                     bufs=2)
                        # staging DMAs ride two queues; int8->fp16 converts
                        # split across scalar and vector engines
                        if j % 2 == 0:
                            nc.sync.dma_start(x8[:],
                                              src[j * 128:(j + 1) * 128, :])
                            nc.scalar.copy(xs[kt][:], x8[:])
                        else:
                            nc.scalar.dma_start(x8[:],
                                                src[j * 128:(j + 1) * 128, :])
                            nc.vector.tensor_copy(xs[kt][:], x8[:])

                # evens-round stash: alpha-scaled fp16 partials [128,32*512]
                stash = invp.tile([128, 32 * FS], FH, tag="stash",
                                  name="stash")
                for g in range(8):
                    pss = [gps.tile([128, FS], DT, tag="gp",
                                    name=f"gpe{g}_{i}", bufs=4)
                           for i in range(4)]
                    for kt in range(0, KT, 2):
                        for i in range(4):
                            bo = g * 4 + i
                            nc.tensor.matmul(
                                pss[i][:],
                                xs[kt][:, bo * 128:(bo + 1) * 128],
                                ws[kt][:], start=(kt == 0), stop=(kt == 14))
                    for i in range(4):
                        bo = g * 4 + i
                        nc.scalar.activation(
                            stash[:, bo * FS:(bo + 1) * FS], pss[i][:],
                            mybir.ActivationFunctionType.Copy,
                            scale=alb[:, 0:1])

                # odds round + inverse pipeline per group
                for g in range(8):
                    pss = [gps.tile([128, FS], DT, tag="gp",
                                    name=f"gpo{g}_{i}", bufs=4)
                           for i in range(4)]
                    for kt in range(1, KT, 2):
                        for i in range(4):
                            bo = g * 4 + i
                            nc.tensor.matmul(
                                pss[i][:],
                                xs[kt][:, bo * 128:(bo + 1) * 128],
                                ws[kt][:], start=(kt == 1), stop=(kt == 15))
                    # yr = alpha*psum + stash  (fused on DVE, fp16 out)
                    yro = invp.tile([128, 4 * FS], FH, tag="yro", name="yro",
                                    bufs=2)
                    for i in range(4):
                        bo = g * 4 + i
                        nc.vector.scalar_tensor_tensor(
                            yro[:, i * FS:(i + 1) * FS], pss[i][:],
                            alb[:, 0:1], stash[:, bo * FS:(bo + 1) * FS],
                            op0=A.mult, op1=A.add)
                    # fused transpose: [b,f] -> [f,b'] with H128 on b
                    uRA = invp.tile([128, 4 * 128 * 4], FH, tag="uRA",
                                    name="uRA", bufs=1)
                    uRB = invp.tile([128, 4 * 128 * 4], FH, tag="uRB",
                                    name="uRB", bufs=1)
                    for i in range(4):
                        psT = gps.tile([128, 512], DT, tag="tp",
                                       name=f"tpt{g}_{i}", bufs=2)
                        for ft in range(4):
                            nc.tensor.matmul(
                                psT[:, ft * 128:(ft + 1) * 128],
                                yro[:, i * FS + ft * 128:
                                    i * FS + (ft + 1) * 128],
                                h128b[:], start=True, stop=True)
                        nc.scalar.copy(uRA[:, i * 512:(i + 1) * 512],
                                       psT[:])
                    # batch-H4 (bo bits 0,1; distance 1,2 in i) on the roll:
                    # layout uRA = [bo-local 4][ft 4][b' 128]
                    for h in range(2):
                        a0, a1 = h * 1024, h * 1024 + 512
                        nc.vector.tensor_tensor(uRB[:, a0:a0 + 512],
                                                uRA[:, a0:a0 + 512],
                                                uRA[:, a1:a1 + 512],
                                                op=A.add)
                        nc.vector.tensor_tensor(uRB[:, a1:a1 + 512],
                                                uRA[:, a0:a0 + 512],
                                                uRA[:, a1:a1 + 512],
                                                op=A.subtract)
                    for h in range(2):
                        a0, a1 = h * 512, h * 512 + 1024
                        nc.vector.tensor_tensor(uRA[:, a0:a0 + 512],
                                                uRB[:, a0:a0 + 512],
                                                uRB[:, a1:a1 + 512],
                                                op=A.add)
                        nc.vector.tensor_tensor(uRA[:, a1:a1 + 512],
                                                uRB[:, a0:a0 + 512],
                                                uRB[:, a1:a1 + 512],
                                                op=A.subtract)
                    # feature H128 per out f-tile; H32/H8 fold into host
                    for ft in range(4):
                        zps = gps.tile([128, 512], DT, tag="zp",
                                       name=f"zps{g}_{ft}", bufs=2)
                        nc.tensor.matmul(
                            zps[:],
                            h128b[:],
                            uRA[:].rearrange("p (bo f b) -> p f bo b",
                                             bo=4, f=4)[:, ft, :, :],
                            start=True, stop=True)
                        ostg = invp.tile([128, 512], FH, tag="ostg",
                                         name=f"ostg{g}_{ft}", bufs=2)
                        nc.scalar.copy(ostg[:], zps[:])
                        nc.sync.dma_start(
                            out[ft * 128:(ft + 1) * 128,
                                g * 512:(g + 1) * 512], ostg[:])
            qsc.release()
    nc.compile()
    return nc


def kernel(**inputs):
    from concourse.bass_utils import run_bass_kernel_spmd

    if "nc" not in _cache:
        _cache["nc"] = _build()
    nc = _cache["nc"]

    x = np.asarray(inputs["inputs"], np.float32).astype(np.float16)
    w = np.asarray(inputs["kernel"], np.float32).astype(np.float16)
    bias = np.asarray(inputs["bias"], np.float32)
    nxp = (0.5 - np.asarray(inputs["noise_x"], np.float32)).astype(np.float16)
    nwp = (0.5 - np.asarray(inputs["noise_w"], np.float32)).astype(np.float16)

    in_maps = []
    for k in range(NCORES):
        cs = slice(k * CS, (k + 1) * CS)
        in_maps.append({
            "xk": np.ascontiguousarray(x[:, cs]),
            "nk": np.ascontiguousarray(nxp[:, cs].T),
            "wk": np.ascontiguousarray(w[cs, :].T),
            "mk": np.ascontiguousarray(nwp[cs, :]),
        })

    res = run_bass_kernel_spmd(nc, in_maps, list(range(NCORES)))
    V = np.stack([np.asarray(r["out"], np.float32)
                  for r in res.results])                   # [a, 4ft*128, B]
    H32 = _sylvester(32)
    H8 = _sylvester(8)
    yT = (H32 @ V.reshape(NCORES * 4, -1)).reshape(F, 8, 512)  # feature H32
    yT = np.einsum('gc,fcb->fgb', H8, yT).reshape(F, B)        # batch H8
    y = np.ascontiguousarray(yT.T) + bias[None, :]
    return y.astype(np.float32)


# revision 9
# speedup vs baseline: 1.3528x; 1.0095x over previous
"""Trainium2 Bass kernel for quantized dense layer with Hadamard rotations.

Math (see reference): y = (H2 @ (sq(H2@x) @ sq(w@H1)) @ H1)/(64*64) + bias,
where sq() is per-tensor symmetric int8 stochastic quantization.

Structure: Sylvester Hadamards factor as Kronecker products
(H4096 = H32 (x) H128).  The forward transform per side is a per-128-tile
fp16 PE matmul against H128 (inputs converted fp32->fp16; validated to
cause ~1.2% stochastic-rounding flips = ~0.45% operand error) plus a
cross-tile DVE butterfly in fp16.  Quantized values (<=127) are exact in
fp16, so the core GEMM runs fp16 x fp16 -> fp32 PSUM exactly.  Stochastic
rounding is rint(x*scale + (0.5 - noise)) via the fp32->int16
round-to-nearest cast, narrowed to int8 for the collectives.

Sharding (8 cores): the IN axis is split 8 ways for forward transforms +
quantization.  v2 schedule: the W side runs FIRST so its scale AllReduce
and the AllToAll land early; the CC stream order is
warmup-AR, AR-w, AR-x, A2A(w int8), AG1(x evens int8), AG2(x odds int8),
so the core GEMM starts as soon as AG1 lands instead of waiting for the
whole chain.  All data collectives ship int8 (half the bytes of fp16);
int8->fp16 conversion rides the scalar/vector engines during the
PE-bound GEMM phase.

The GEMM runs in two rounds: evens k-tiles accumulate while AG2 is in
flight and are stashed to SBUF as alpha-scaled fp16; the odds round adds
the stash back via a fused scalar_tensor_tensor.  The inverse fuses the
batch H128 into the post-GEMM PE transpose and applies the batch H4
(bo bits 0-1) as a DVE roll; the feature H128 is one PE matmul per
f-tile.  The remaining inverse factors (feature H32 over core x f-tile,
batch H8 over bo-chunk) fold into the host-side unshard combine.

Known hardware behaviors factored in: PE HAM throttle (1.2 GHz cold /
2.4 GHz after ~3.4us sustained); fp32 matmuls lower to 2 half-speed
passes -- avoid; collectives serialize on one CC stream (emission order
= stream order) with ~10-30us latency floors; a warmup AllReduce at t=0
absorbs the CC-entry barrier and inter-core launch skew; scalar-engine
copies offload PSUM evacuation.
"""
import sys, os
sys.path.insert(0, '/opt/trn_rl_repo')
import numpy as np

B, IN, F = 4096, 2048, 4096
NCORES = 8
CS = IN // NCORES      # 256  per-core IN slice
FS = F // NCORES       # 512  per-core feature block
BT = B // 128          # 32   batch tiles
KT = IN // 128         # 16   contraction tiles
QMAX = 127.0

_cache = {}


def _sylvester(n):
    h = np.array([[1.0]], dtype=np.float32)
    while h.shape[0] < n:
        h = np.block([[h, h], [h, -h]])
    return h


def _build():
    from concourse import bass, bacc, tile, mybir
    import concourse.bass_isa as bass_isa

    DT = mybir.dt.float32
    FH = mybir.dt.float16
    I16 = mybir.dt.int16
    I8 = mybir.dt.int8
    A = mybir.AluOpType
    nph = np.float16

    nc = bacc.Bacc("TRN2", target_bir_lowering=False, debug=False,
                   num_devices=NCORES)

    xk = nc.dram_tensor("xk", [B, CS], FH, kind="ExternalInput")
    nk = nc.dram_tensor("nk", [CS, B], FH, kind="ExternalInput")   # (0.5-noise_x)^T
    wk = nc.dram_tensor("wk", [F, CS], FH, kind="ExternalInput")   # w slice^T
    mk = nc.dram_tensor("mk", [CS, F], FH, kind="ExternalInput")   # 0.5-noise_w
    out = nc.dram_tensor("out", [FS, B], FH, kind="ExternalOutput")

    wu_i = nc.dram_tensor("wu_i", [1, 1], DT)
    wu_o = nc.dram_tensor("wu_o", [1, 1], DT, addr_space="Shared")
    s2_i = nc.dram_tensor("s2_i", [1, 2], DT)
    s2_o = nc.dram_tensor("s2_o", [1, 2], DT, addr_space="Shared")
    xqc0 = nc.dram_tensor("xqc0", [128, B], I8)                    # xq^T k-half 0
    xqc1 = nc.dram_tensor("xqc1", [128, B], I8)                    # xq^T k-half 1
    xqg0 = nc.dram_tensor("xqg0", [NCORES * 128, B], I8, addr_space="Shared")
    xqg1 = nc.dram_tensor("xqg1", [NCORES * 128, B], I8, addr_space="Shared")
    wac = nc.dram_tensor("wac", [IN, FS], I8)                      # A2A contrib
    wblk = nc.dram_tensor("wblk", [IN, FS], I8)

    h128b_d = nc.inline_tensor(_sylvester(128).astype(nph), name="h128b")
    h128n_d = nc.inline_tensor((-_sylvester(128)).astype(nph), name="h128n")
    idb_d = nc.inline_tensor(np.eye(128, dtype=nph), name="idb")
    H4 = _sylvester(4)
    rg = [list(range(NCORES))]

    NB = 32 * CS  # 8192 free columns in a fwd big tile

    def butterfly(nc, bufs, T, blk0, A):
        """FWHT across the tile-index axis of big tensors [128, T*blk0]."""
        n = T.bit_length() - 1
        for s in range(n):
            cur, nxt = bufs(s)
            blk = blk0 << s
            hi = T >> (s + 1)
            for h in range(hi):
                a0 = h * 2 * blk
                a1 = a0 + blk
                nc.vector.tensor_tensor(nxt[:, a0:a0 + blk],
                                        cur[:, a0:a0 + blk],
                                        cur[:, a1:a1 + blk], op=A.add)
                nc.vector.tensor_tensor(nxt[:, a1:a1 + blk],
                                        cur[:, a0:a0 + blk],
                                        cur[:, a1:a1 + blk], op=A.subtract)

    with tile.TileContext(nc) as tc:
        with tc.tile_pool(name="consts", bufs=1) as cpool:
            h128b = cpool.tile([128, 128], FH)
            h128n = cpool.tile([128, 128], FH)
            idb = cpool.tile([128, 128], FH)
            nc.sync.dma_start(h128b[:], h128b_d[:])
            nc.sync.dma_start(h128n[:], h128n_d[:])
            nc.sync.dma_start(idb[:], idb_d[:])
            qsc = tc.alloc_tile_pool(name="qsc", bufs=1)
            wu = qsc.tile([1, 1], DT, tag="wu", name="wu")
            nc.vector.memset(wu[0:1, 0:1], 0.0)
            nc.sync.dma_start(wu_i[:], wu[0:1, 0:1])
            nc.gpsimd.collective_compute(
                "AllReduce", A.max, replica_groups=rg,
                ins=[wu_i.ap().opt()], outs=[wu_o.ap().opt()])

            # ================= forward transforms + quant =================
            with tc.tile_pool(name="fwd", bufs=2) as fp_, \
                 tc.tile_pool(name="fin", bufs=4) as fin, \
                 tc.tile_pool(name="fps", bufs=1, space="PSUM") as fps, \
                 tc.tile_pool(name="qtmp", bufs=2) as qtmp, \
                 tc.tile_pool(name="qT", bufs=3) as qTp:

                def fwd_side(src_tile_ap, ntiles, side):
                    bigA = fp_.tile([128, NB], FH, tag="bigA",
                                    name=f"bigA{side}")
                    bigB = fp_.tile([128, NB], FH, tag="bigB",
                                    name=f"bigB{side}")
                    # H128 (x) H4: per 4-tile group, each output tile is a
                    # 4-term +/-H128 PSUM accumulation (DVE TT runs at 1x
                    # mode, so trading 2 butterfly stages for PE matmuls
                    # wins; the PE load also warms the HAM clock early)
                    for g4 in range(ntiles // 4):
                        ths = []
                        for m in range(4):
                            th = fin.tile([128, CS], FH, tag="finh",
                                          name="finth", bufs=10)
                            nc.sync.dma_start(th[:],
                                              src_tile_ap(g4 * 4 + m))
                            ths.append(th)
                        for mp in range(4):
                            o = g4 * 4 + mp
                            ps = fps.tile([128, CS], DT, tag="ps",
                                          name="fpst", bufs=4)
                            for m in range(4):
                                st = h128b if H4[mp, m] > 0 else h128n
                                nc.tensor.matmul(ps[:], st[:], ths[m][:],
                                                 start=(m == 0),
                                                 stop=(m == 3))
                            # PSUM->SBUF copies on the scalar engine
                            nc.scalar.copy(bigA[:, o * CS:(o + 1) * CS],
                                           ps[:])
                    bufs = (lambda s: (bigA, bigB) if s % 2 == 0
                            else (bigB, bigA))
                    butterfly(nc, bufs, 8, 4 * CS, A)
                    return bigB  # 3 stages -> result in B

                def scale_trigger(big, tag, col):
                    # abs-max reduce of the whole tile; GPSIMD cross-partition
                    # reduce frees DVE for the next butterfly.
                    am = qsc.tile([128, 1], DT, tag=f"am{tag}",
                                  name=f"am{tag}")
                    nc.vector.tensor_reduce(am[:], big[:],
                                            axis=mybir.AxisListType.X,
                                            op=A.max,
                                            apply_absolute_value=True)
                    red = qsc.tile([128, 1], DT, tag=f"rd{tag}",
                                   name=f"rd{tag}")
                    nc.gpsimd.partition_all_reduce(
                        red[:], am[:], channels=128,
                        reduce_op=bass_isa.ReduceOp.absmax)
                    nc.sync.dma_start(s2_i[0:1, col:col + 1], red[0:1, 0:1])

                def scale_finish(tag, col):
                    sg = qsc.tile([1, 1], DT, tag=f"sg{tag}",
                                  name=f"sg{tag}")
                    nc.sync.dma_start(sg[0:1, :], s2_o[0:1, col:col + 1])
                    # r = QMAX/s (hardware iterative divide is accurate; a
                    # scale off by 2^-23 shifts ~no stochastic decisions)
                    r0 = qsc.tile([1, 1], DT, tag=f"r0{tag}", name=f"r0{tag}")
                    nc.vector.reciprocal(r0[0:1, :], sg[0:1, :])
                    r127 = qsc.tile([1, 1], DT, tag=f"rq{tag}",
                                    name=f"rq{tag}")
                    nc.vector.tensor_scalar_mul(r127[0:1, :], r0[0:1, :],
                                                QMAX)
                    rb = qsc.tile([128, 1], DT, tag=f"rb{tag}",
                                  name=f"rb{tag}")
                    nc.gpsimd.partition_broadcast(rb[:, 0:1], r127[0:1, 0:1])
                    return sg, rb

                def pre_transpose(big, ntiles, side):
                    """PE-transpose the rotated fp16 data [128, ntiles*CS]
                    into two k-half tiles [128, ntiles*128]; 4 blocks batch
                    into one PSUM tile so evacuation is 4x cheaper."""
                    outs = [qTp.tile([128, ntiles * 128], FH, tag="qT",
                                     name=f"{side}T{h}", bufs=4)
                            for h in range(2)]
                    for h in range(2):
                        for o4 in range(ntiles // 4):
                            ps = fps.tile([128, 512], FH, tag="tps",
                                          name="tpst", bufs=4)
                            for j in range(4):
                                o = o4 * 4 + j
                                nc.tensor.transpose(
                                    ps[:, j * 128:(j + 1) * 128],
                                    big[:, o * CS + h * 128:o * CS +
                                        (h + 1) * 128], idb[:])
                            nc.scalar.copy(
                                outs[h][:, o4 * 512:(o4 + 1) * 512], ps[:])
                    return outs

                def quant_half(tT, rb, nz, side):
                    """stochastic-round one k-half [128, N] in final layout:
                    STT -> int16 (rint via cast), narrow -> int8."""
                    n = tT.shape[1]
                    qi = qtmp.tile([128, n], I16, tag="qi", name="qit",
                                   bufs=2)
                    nc.vector.scalar_tensor_tensor(
                        qi[:], tT[:], rb[:, 0:1], nz[:], op0=A.mult,
                        op1=A.add)
                    qh = qtmp.tile([128, n], I8, tag="qh", name="qht",
                                   bufs=4)
                    nc.scalar.copy(qh[:], qi[:])
                    return qh

                # ---- w side first: fwd + AR-w + quant + A2A ----
                nzw = [qtmp.tile([128, F], FH, tag="nzw", name=f"nzw{h}",
                                 bufs=2) for h in range(2)]
                for h in range(2):
                    nc.scalar.dma_start(nzw[h][:],
                                        mk[h * 128:(h + 1) * 128, :])
                wrB = fwd_side(lambda o: wk[o * 128:(o + 1) * 128, :],
                               F // 128, "w")
                scale_trigger(wrB, "w", 1)

                nzx = [qtmp.tile([128, B], FH, tag="nzx", name=f"nzx{h}",
                                 bufs=2) for h in range(2)]
                for h in range(2):
                    nc.scalar.dma_start(nzx[h][:],
                                        nk[h * 128:(h + 1) * 128, :])

                # ---- x side fwd (DVE butterfly overlaps AR-w flight);
                # emitted before pre_transpose(w) so the x H128 matmuls
                # aren't stuck on the PE FIFO behind transposes that wait
                # for the w butterfly ----
                xrB = fwd_side(lambda o: xk[o * 128:(o + 1) * 128, :], BT,
                               "x")
                scale_trigger(xrB, "x", 0)
                # ONE AllReduce for both scales (saves a ~20us stream slot)
                nc.gpsimd.collective_compute(
                    "AllReduce", A.max, replica_groups=rg,
                    ins=[s2_i.ap().opt()], outs=[s2_o.ap().opt()])

                wrT = pre_transpose(wrB, F // 128, "w")
                xrT = pre_transpose(xrB, BT, "x")

                # x quant -> AG1 (evens = k-half 0) first on the stream
                sgx, rbx = scale_finish("x", 0)
                sgw, rbw = scale_finish("w", 1)
                qh0 = quant_half(xrT[0], rbx, nzx[0], "x")
                nc.sync.dma_start(xqc0[:, :], qh0[:])
                nc.gpsimd.collective_compute(
                    "AllGather", A.bypass, replica_groups=rg,
                    ins=[xqc0.ap().opt()], outs=[xqg0.ap().opt()])

                # w quant -> A2A
                wqh = [quant_half(wrT[h], rbw, nzw[h], "w")
                       for h in range(2)]
                for a in range(NCORES):
                    for h in range(2):
                        nc.sync.dma_start(
                            wac[a * CS + h * 128:a * CS + (h + 1) * 128, :],
                            wqh[h][:, a * FS:(a + 1) * FS])
                nc.gpsimd.collective_compute(
                    "AllToAll", A.bypass, replica_groups=rg,
                    ins=[wac.ap().opt()], outs=[wblk.ap().opt()])

                # x odds -> AG2
                qh1 = quant_half(xrT[1], rbx, nzx[1], "x")
                nc.sync.dma_start(xqc1[:, :], qh1[:])
                nc.gpsimd.collective_compute(
                    "AllGather", A.bypass, replica_groups=rg,
                    ins=[xqc1.ap().opt()], outs=[xqg1.ap().opt()])

                # alpha = sx*sw/(QMAX^2 * 2^24)
                al = qsc.tile([1, 1], DT, tag="al", name="al")
                nc.vector.tensor_tensor(al[0:1, 0:1], sgx[0:1, 0:1],
                                        sgw[0:1, 0:1], op=A.mult)
                nc.vector.tensor_scalar_mul(
                    al[0:1, 0:1], al[0:1, 0:1],
                    float(1.0 / (QMAX * QMAX * (1 << 24))))
                alb = qsc.tile([128, 1], DT, tag="alb", name="alb")
                nc.gpsimd.partition_broadcast(alb[:, 0:1], al[0:1, 0:1])

            # ================= GEMM + fused inverse =================
            with tc.tile_pool(name="gem", bufs=1) as gem, \
                 tc.tile_pool(name="g8", bufs=2) as g8, \
                 tc.tile_pool(name="gps", bufs=1, space="PSUM") as gps, \
                 tc.tile_pool(name="inv", bufs=1) as invp:
                # int8 staging rotates; fp16 tiles persist through the GEMM
                xs = [gem.tile([128, B], FH, tag="xs", name=f"xst{kt}",
                               bufs=KT) for kt in range(KT)]
                ws = [gem.tile([128, FS], FH, tag="ws", name=f"wst{kt}",
                               bufs=KT) for kt in range(KT)]
                for j in range(NCORES):   # xs evens: first on every FIFO
                    kt = 2 * j
                    x8 = g8.tile([128, B], I8, tag="x8", name=f"x8_{kt}",
                                 bufs=2)
                    nc.sync.dma_start(x8[:], xqg0[j * 128:(j + 1) * 128, :])
                    if j % 2 == 0:
                        nc.scalar.copy(xs[kt][:], x8[:])
                    else:
                        nc.vector.tensor_copy(xs[kt][:], x8[:])
                for kt in range(KT):      # ws: scalar converts (behind A2A)
                    w8 = g8.tile([128, FS], I8, tag="w8", name=f"w8_{kt}",
                                 bufs=4)
                    nc.scalar.dma_start(w8[:],
                                        wblk[kt * 128:(kt + 1) * 128, :])
                    nc.scalar.copy(ws[kt][:], w8[:])
                for j in range(NCORES):   # xs odds: vector converts (AG2)
                    kt = 2 * j + 1
                    x8 = g8.tile([128, B], I8, tag="x8", name=f"x8_{kt}",
                                 bufs=2)
                    nc.sync.dma_start(x8[:], xqg1[j * 128:(j + 1) * 128, :])
                    nc.vector.tensor_copy(xs[kt][:], x8[:])

                # evens-round stash: alpha-scaled fp16 partials [128,32*512]
                stash = invp.tile([128, 32 * FS], FH, tag="stash",
                                  name="stash")
                for g in range(8):
                    pss = [gps.tile([128, FS], DT, tag="gp",
                                    name=f"gpe{g}_{i}", bufs=4)
                           for i in range(4)]
                    for kt in range(0, KT, 2):
                        for i in range(4):
                            bo = g * 4 + i
                            nc.tensor.matmul(
                                pss[i][:],
                                xs[kt][:, bo * 128:(bo + 1) * 128],
                                ws[kt][:], start=(kt == 0), stop=(kt == 14))
                    for i in range(4):
                        bo = g * 4 + i
                        nc.scalar.activation(
                            stash[:, bo * FS:(bo + 1) * FS], pss[i][:],
                            mybir.ActivationFunctionType.Copy,
                            scale=alb[:, 0:1])

                # odds round + inverse pipeline per group
                for g in range(8):
                    pss = [gps.tile([128, FS], DT, tag="gp",
                                    name=f"gpo{g}_{i}", bufs=4)
                           for i in range(4)]
                    for kt in range(1, KT, 2):
                        for i in range(4):
                            bo = g * 4 + i
                            nc.tensor.matmul(
                                pss[i][:],
                                xs[kt][:, bo * 128:(bo + 1) * 128],
                                ws[kt][:], start=(kt == 1), stop=(kt == 15))
                    # yr = alpha*psum + stash  (fused on DVE, fp16 out)
                    yro = invp.tile([128, 4 * FS], FH, tag="yro", name="yro",
                                    bufs=2)
                    for i in range(4):
                        bo = g * 4 + i
                        nc.vector.scalar_tensor_tensor(
                            yro[:, i * FS:(i + 1) * FS], pss[i][:],
                            alb[:, 0:1], stash[:, bo * FS:(bo + 1) * FS],
                            op0=A.mult, op1=A.add)
                    # fused transpose: [b,f] -> [f,b'] with H128 on b
                    uRA = invp.tile([128, 4 * 128 * 4], FH, tag="uRA",
                                    name="uRA", bufs=1)
                    uRB = invp.tile([128, 4 * 128 * 4], FH, tag="uRB",
                                    name="uRB", bufs=1)
                    for i in range(4):
                        psT = gps.tile([128, 512], DT, tag="tp",
                                       name=f"tpt{g}_{i}", bufs=2)
                        for ft in range(4):
                            nc.tensor.matmul(
                                psT[:, ft * 128:(ft + 1) * 128],
                                yro[:, i * FS + ft * 128:
                                    i * FS + (ft + 1) * 128],
                                h128b[:], start=True, stop=True)
                        nc.scalar.copy(uRA[:, i * 512:(i + 1) * 512],
                                       psT[:])
                    # batch-H4 (bo bits 0,1; distance 1,2 in i) on the roll:
                    # layout uRA = [bo-local 4][ft 4][b' 128]
                    for h in range(2):
                        a0, a1 = h * 1024, h * 1024 + 512
                        nc.vector.tensor_tensor(uRB[:, a0:a0 + 512],
                                                uRA[:, a0:a0 + 512],
                                                uRA[:, a1:a1 + 512],
                                                op=A.add)
                        nc.vector.tensor_tensor(uRB[:, a1:a1 + 512],
                                                uRA[:, a0:a0 + 512],
                                                uRA[:, a1:a1 + 512],
                                                op=A.subtract)
                    for h in range(2):
                        a0, a1 = h * 512, h * 512 + 1024
                        nc.vector.tensor_tensor(uRA[:, a0:a0 + 512],
                                                uRB[:, a0:a0 + 512],
                                                uRB[:, a1:a1 + 512],
                                                op=A.add)
                        nc.vector.tensor_tensor(uRA[:, a1:a1 + 512],
                                                uRB[:, a0:a0 + 512],
                                                uRB[:, a1:a1 + 512],
                                                op=A.subtract)
                    # feature H128 per out f-tile; H32/H8 fold into host
                    for ft in range(4):
                        zps = gps.tile([128, 512], DT, tag="zp",
                                       name=f"zps{g}_{ft}", bufs=2)
                        nc.tensor.matmul(
                            zps[:],
                            h128b[:],
                            uRA[:].rearrange("p (bo f b) -> p f bo b",
                                             bo=4, f=4)[:, ft, :, :],
                            start=True, stop=True)
                        ostg = invp.tile([128, 512], FH, tag="ostg",
                                         name=f"ostg{g}_{ft}", bufs=2)
                        nc.scalar.copy(ostg[:], zps[:])
                        nc.sync.dma_start(
                            out[ft * 128:(ft + 1) * 128,
                                g * 512:(g + 1) * 512], ostg[:])
            qsc.release()
    nc.compile()
    return nc


def kernel(**inputs):
    from concourse.bass_utils import run_bass_kernel_spmd

    if "nc" not in _cache:
        _cache["nc"] = _build()
    nc = _cache["nc"]

    x = np.asarray(inputs["inputs"], np.float32).astype(np.float16)
    w = np.asarray(inputs["kernel"], np.float32).astype(np.float16)
    bias = np.asarray(inputs["bias"], np.float32)
    nxp = (0.5 - np.asarray(inputs["noise_x"], np.float32)).astype(np.float16)
    nwp = (0.5 - np.asarray(inputs["noise_w"], np.float32)).astype(np.float16)

    in_maps = []
    for k in range(NCORES):
        cs = slice(k * CS, (k + 1) * CS)
        in_maps.append({
            "xk": np.ascontiguousarray(x[:, cs]),
            "nk": np.ascontiguousarray(nxp[:, cs].T),
            "wk": np.ascontiguousarray(w[cs, :].T),
            "mk": np.ascontiguousarray(nwp[cs, :]),
        })

    res = run_bass_kernel_spmd(nc, in_maps, list(range(NCORES)))
    V = np.stack([np.asarray(r["out"], np.float32)
                  for r in res.results])                   # [a, 4ft*128, B]
    H32 = _sylvester(32)
    H8 = _sylvester(8)
    yT = (H32 @ V.reshape(NCORES * 4, -1)).reshape(F, 8, 512)  # feature H32
    yT = np.einsum('gc,fcb->fgb', H8, yT).reshape(F, B)        # batch H8
    y = np.ascontiguousarray(yT.T) + bias[None, :]
    return y.astype(np.float32)
